# revision 99
# baseline (speedup 1.0000x reference)
"""Trainium2 Bass kernel for nn_CrossAttention_55130200212194.

Sharding: head h -> core h (8 heads, 8 cores, one replicated NEFF; cores
differ only in input data).  Host prep = layout/dtype only (transposes,
bf16 casts, constant prescales); every FLOP of the module runs on device.
Host combine = sum of the 8 partial [2048,640] projections (column-
sharded Wout, bf16 partials).

All-bf16 data path.  fp8/DoubleRow and Schraudolph-exp variants were
measured and rejected: the output is an attention-weighted average, so
multiplicative quantization noise passes through at full relative
strength (any single fp8 stage costs ~2e-2 max-rel-err vs the 2e-2
gate; bf16 lands at 4.2e-3 end to end).

  - scores: 3 bf16 matmuls per j-tile (K=80; bf16 runs 1 PE cyc/row at
    any free size), gamma/SCALE folded into the host k prescales, all
    three k tensors packed in one DMA tensor.
  - exp: exact, ACT-only (the 66us floor of this kernel), one op per
    j-tile over a 2-bank PSUM pair: mix+self paths share one [P,2,IC]
    tile and one shift of -4.5 (self logits reach 8.47; the shift
    cancels in the softmax ratio) -> bf16 em/es pair tile.
  - attn@v in [i,d] output orientation: em/es [j,i] slices are the
    STATIONARY operand (weight loads are free in the cost model), v
    extended to 81 cols (col 80 = 1/gamma resp. 1/beta) is the moving
    operand: 4x81 cyc per j-tile instead of 2x512.  Z/gamma lands as a
    per-partition COLUMN; 1/Z via single-op reciprocal_approx_fast;
    normalization is two per-partition-scalar DVE ops (no broadcast
    matmuls, no one-hot weights).
  - merged [i,d] tiles land in per-slot mg tiles (128 cols, col 96 =
    1.0; separate tiles because the Tile scheduler serializes
    cross-engine writes to one tile at tile granularity), transpose to
    [d,i] via DMA-crossbar mid-loop (HWDGE is idle there) and via PE
    is_transpose matmuls at the tail; either way the transpose plants
    the ones-row at row 96 that pairs with WoT's bias row (core 0
    only), so the projection bias is free.
  - software pipelining: attn@v lags scores/exp by FOUR j-tiles
    (em ring bufs=6), which keeps ACT back-to-back at its 1038ns/j-tile
    floor; qc/v_self prologue GEMMs and output projections interleave
    into the j-loop on a shared 2-bank PSUM tag (projects at j=3,6,9,12
    reading the previous chunk's merges from j=3; qc for the next
    chunk at j=14); early k3/xT DMA slices split so the first 3 j-tiles
    aren't gated on bulk transfers.
  - tail: j-major flush of the 4 pending attn@v tiles, batched 8-way
    reciprocal, DVE normalize, then per-s PRIVATE psum regions (sc ring
    x2 + aux + freed oDS banks) for transpose+projection so nothing
    serializes on buffer reuse; fsb evacs alternate DVE/ACT whole-tile
    (single-engine per tile, same cross-engine-write rule).
  - outputs stored bf16 (halves output DMA bytes; host sums as f32).

Timeline (cost model, per core): 102.4us baseline -> 91.5us.  The
score matmuls run under tc.high_priority(offset=64) for ic>0, j>1 —
biasing them ahead of same-cadence attn@v/aux work in the scheduler's
PE ready heap is worth ~0.5us.  Priority sensitivity map (measured):
boosting attn@v +3.6us, boosting qc +0.9us, deprioritizing projects
+1.4us, scope ic>1 +0.3us; priorities on dependency-bound stages
(tail merges/evacs, mid-loop out-DMAs) are neutral.  ACT is the
floor: 64 exps x (853+185)ns = 66.4us busy.  Mid-loop PE runs at
exact capacity (scores 640 + attn@v 270 + amortized qc/projects 133
= 1,043ns/j vs the 1,038 exp cadence), so hiccups never re-absorb:
that zero-slack balance is the ~1us/chunk residual.  Remaining idle:
8.3us startup chain (xT0 DMA -> 5-step qc -> evac -> dual score),
~7.8us post-last-exp drain (flush/merge/transpose/project/evac/DMA +
900ns DMA-sem + barriers); the final store is split lo/hi across
ACT/DVE with two DMAs so the gating transfer is the small 128-col
one.  fp8/DoubleRow self-path scores
were re-measured: 2.6e-2 rel err vs the 2e-2 gate -> rejected (bf16
lands at 4.2e-3).  qc in flipped [i,d] orientation (1600 vs 2560 PE
cyc/chunk) with DMA-crossbar transpose back: +2.4us measured — the
transpose latency forces qc to j=10, disrupting the aux ring and the
project rhythm; PE busy (70.1us) exceeds the ACT exp stream (66.4us),
so the mid-loop is PE-floor-bound and no remaining PE cut survives
its own induced staging traffic.  The Tile scheduler re-derives instruction order by
simulating with its own cost model, so waits quantize to engine counts
at emission and pure emission reordering is usually neutral; the wins
that survive are real dependency/latency changes (lag depth, DMA
splits, private psum regions, per-slot staging tiles).
"""

import os
import sys

sys.path.insert(0, "/opt/trn_rl_repo")

import numpy as np
import ml_dtypes

H = 8
N = 2048
D = 80
C = 640
SCALE = D ** -0.5
GAMMA = 0.7
BETA = 0.3
P = 128
IC = 512                 # i-chunk (PSUM bank = 512 fp32)
NJT = N // P             # 16 j-tiles
NICH = N // IC           # 4 i-chunks
NSUB = IC // P           # 4 i-subtiles per chunk
VE = 81                  # v extended cols: 80 d + 1/w col
NCORES = 8

SHIFT = 4.5              # unified logit shift (self logits reach 8.47)

BF16 = ml_dtypes.bfloat16

_CACHE = {}
LAST_EXEC_NS = None


def _build_nc():
    from contextlib import nullcontext

    import concourse.mybir as mybir
    import concourse.tile as tile
    from concourse import bacc
    from concourse.bass import ts

    f32 = mybir.dt.float32
    bf16 = mybir.dt.bfloat16
    Exp = mybir.ActivationFunctionType.Exp
    Alu = mybir.AluOpType

    nc = bacc.Bacc(
        "TRN2",
        target_bir_lowering=False,
        debug=False,
        enable_asserts=False,
        num_devices=NCORES,
    )

    xT_d = nc.dram_tensor("xT", [P, 5, N], bf16, kind="ExternalInput")
    qiT_d = nc.dram_tensor("qiT", [D, N], bf16, kind="ExternalInput")
    k3_d = nc.dram_tensor("k3", [D, 3, N], bf16, kind="ExternalInput")
    ve_d = nc.dram_tensor("ve", [P, NJT, VE], bf16, kind="ExternalInput")
    WqT_d = nc.dram_tensor("WqT", [P, 5, D], bf16, kind="ExternalInput")
    WvT_d = nc.dram_tensor("WvT", [P, 5, D], bf16, kind="ExternalInput")
    WoT_d = nc.dram_tensor("WoT", [P, C], bf16, kind="ExternalInput")
    ident_d = nc.dram_tensor("ident", [P, P], bf16, kind="ExternalInput")
    out_d = nc.dram_tensor("out", [N, C], bf16, kind="ExternalOutput")

    with tile.TileContext(nc) as tc:
        with (
            tc.tile_pool(name="const", bufs=1) as const,
            tc.tile_pool(name="work", bufs=2) as work,
            tc.tile_pool(name="fout", bufs=3) as fout,
            tc.tile_pool(name="psum", bufs=1, space="PSUM") as pm,
        ):
            xT = const.tile([P, 5, N], bf16, tag="xT")
            qiT = const.tile([P, N], bf16, tag="qiT")
            qcT = const.tile([P, N], bf16, tag="qcT")
            k3 = const.tile([P, 3, N], bf16, tag="k3")
            v_e = const.tile([P, NJT, VE], bf16, tag="v_e")
            vs_e = const.tile([P, NJT, VE], bf16, tag="vs_e")
            WqT = const.tile([P, 5, D], bf16, tag="WqT")
            WvT = const.tile([P, 5, D], bf16, tag="WvT")
            WoT = const.tile([P, C], bf16, tag="WoT")
            mergedT = const.tile([P, N], bf16, tag="mergedT")
            # merged [i,d] staging: 8 slots (chunk parity x 4 i-subs),
            # cols 80:128 zero except col 96 = 1.0: the DMA transpose
            # plants mergedT's ones bias-row (96) + zero rows for free.
            mgs = [const.tile([P, P], bf16, tag=f"mg{k}", name=f"mg{k}")
                   for k in range(2 * NSUB)]
            ident = const.tile([P, P], bf16, tag="ident")
            nbias = const.tile([P, 1], f32, tag="nbias")

            nc.gpsimd.memset(nbias[:], -SHIFT)
            for t in mgs:
                nc.gpsimd.memset(t[:], 0.0)
                nc.gpsimd.memset(t[:, 96:97], 1.0)
            nc.gpsimd.memset(vs_e[:, :, D:VE], 0.0)
            nc.gpsimd.memset(vs_e[:, :, D : D + 1], 1.0 / BETA)
            # prefetch the Exp table during the input DMA window
            nc.scalar.activation(
                nbias[0:1, 0:1],
                nbias[0:1, 0:1],
                Exp,
                bias=nbias[0:1, 0:1],
                scale=0.0,
            )
            nc.gpsimd.memset(nbias[:], -SHIFT)

            # ---- DMAs in consumer-priority order (first-use order; the
            # sim serializes all queues on one DMA device token) ----
            nc.sync.dma_start(WqT[:], WqT_d.ap())
            nc.sync.dma_start(xT[:, :, ts(0, IC)], xT_d.ap()[:, :, ts(0, IC)])
            nc.sync.dma_start(qiT[0:D, ts(0, IC)], qiT_d.ap()[:, ts(0, IC)])
            # all three k tensors in one tensor; j-tiles 0-1 first
            nc.sync.dma_start(k3[0:D, :, 0:256], k3_d.ap()[:, :, 0:256])
            nc.sync.dma_start(k3[0:D, :, 256:768], k3_d.ap()[:, :, 256:768])
            nc.sync.dma_start(WvT[:], WvT_d.ap())
            nc.sync.dma_start(v_e[:], ve_d.ap())
            nc.sync.dma_start(
                k3[0:D, :, 768:N], k3_d.ap()[:, :, 768:N]
            )
            nc.sync.dma_start(qiT[0:D, IC:N], qiT_d.ap()[:, IC:N])
            for ic in range(1, NICH):
                nc.sync.dma_start(
                    xT[:, :, ts(ic, IC)], xT_d.ap()[:, :, ts(ic, IC)]
                )
            nc.sync.dma_start(WoT[:], WoT_d.ap())
            nc.sync.dma_start(ident[:], ident_d.ap())

            # shared 2-bank aux tag for qc / v_self / projections
            def aux_tile():
                return pm.tile([P, 2 * IC], f32, tag="fin", bufs=1,
                               name="aux")

            def qc_block(ic):
                qc_part(ic, (0, 1, 2, 3, 4))

            qc_live = {}

            def qc_part(ic, cs):
                # split qc across iterations: 5 x 213ns matmuls overflow a
                # single cadence and delay the next chunk's first scores
                if 0 in cs:
                    qc_live["t"] = aux_tile()
                qps = qc_live["t"]
                for c in cs:
                    nc.tensor.matmul(
                        qps[0:D, 0:IC], WqT[:, c, :], xT[:, c, ts(ic, IC)],
                        start=(c == 0), stop=(c == 4),
                        skip_group_check=True,
                    )
                if 4 in cs:
                    with tc.high_priority(offset=64):
                        nc.vector.tensor_copy(
                            qcT[0:D, ts(ic, IC)], qps[0:D, 0:IC]
                        )

            def vself_quad(q):
                # n-tiles 4q..4q+3, 256-fp32 slots; bank starts at k 0 / 2
                psv = aux_tile()
                for k in range(4):
                    t = 4 * q + k
                    for c in range(5):
                        nc.tensor.matmul(
                            psv[:, k * 256 : k * 256 + D],
                            xT[:, c, ts(t, P)], WvT[:, c, :],
                            start=(c == 0 and k % 2 == 0),
                            stop=(c == 4),
                            skip_group_check=True,
                        )
                for k in range(4):
                    t = 4 * q + k
                    nc.vector.tensor_copy(
                        vs_e[:, t, 0:D], psv[:, k * 256 : k * 256 + D]
                    )

            def project(pic, t):
                nt = 4 * pic + t
                fin = aux_tile()
                nc.tensor.matmul(
                    fin[:, 0:IC], mergedT[:, ts(nt, P)], WoT[:, 0:IC],
                    start=True, stop=True, skip_group_check=True,
                )
                nc.tensor.matmul(
                    fin[:, IC:C], mergedT[:, ts(nt, P)], WoT[:, IC:C],
                    start=True, stop=True, skip_group_check=True,
                )
                fsb = fout.tile([P, C], bf16, tag="fsb", bufs=4)
                # the evac frees the shared aux psum for the next user:
                # bias it ahead of other ready DVE work
                with tc.high_priority(offset=64):
                    nc.vector.tensor_copy(fsb[:], fin[:, 0:C])
                # the store gates nothing mid-loop: yield the shared
                # SP/HWDGE path to the project-gating DMA transposes
                with tc.high_priority(offset=-64):
                    nc.sync.dma_start(out_d.ap()[ts(nt, P), :], fsb[:])

            def merge_isub(pic, oDS, s):
                slot = (pic % 2) * NSUB + s
                rcol = work.tile([P, 2], f32, tag="rc", bufs=8)
                nc.vector.reciprocal_approx_fast(
                    out=rcol[:, 0:1], in_=oDS[:, 0, s, D : D + 1]
                )
                nc.vector.reciprocal_approx_fast(
                    out=rcol[:, 1:2], in_=oDS[:, 1, s, D : D + 1]
                )
                nc.vector.tensor_scalar(
                    mgs[slot][:, 0:D], oDS[:, 0, s, 0:D],
                    rcol[:, 0:1], None, Alu.mult,
                )
                nc.vector.scalar_tensor_tensor(
                    mgs[slot][:, 0:D], oDS[:, 1, s, 0:D],
                    rcol[:, 1:2], mgs[slot][:, 0:D],
                    Alu.mult, Alu.add,
                )
                nc.sync.dma_start_transpose(
                    mergedT[:, ts(NSUB * pic + s, P)], mgs[slot][:]
                )

            # ---- fused prologue + main loop, software-pipelined ----
            qc_block(0)

            pend = []            # (emes, j, oDS) pending attn@v, depth 2
            late_merge = []
            for ic in range(NICH):
                win = ts(ic, IC)
                # [P, slot(D/S), isub, 128]: slot0 = bank0, slot1 = bank1
                oDS = pm.tile([P, 2, NSUB, P], f32, tag="o", bufs=1)
                for j in range(NJT):
                    # second half of the previous chunk's merges, deferred
                    # one iteration so the DVE queue doesn't lock the aux
                    # ring for two cadences at the boundary
                    if late_merge:
                        lic, lDS = late_merge.pop()
                        for s in (2, 3):
                            merge_isub(lic, lDS, s)
                    sc = pm.tile([P, 2, IC], f32, tag="sc", bufs=2)
                    # scores feed the ACT-critical exp: bias them ahead of
                    # same-cadence attn@v / aux matmuls in the PE ready heap
                    with tc.high_priority(offset=64):
                        nc.tensor.matmul(
                            sc[:, 0, :], k3[0:D, 0, ts(j, P)], qiT[0:D, win],
                            start=True, stop=False,
                        )
                        nc.tensor.matmul(
                            sc[:, 0, :], k3[0:D, 2, ts(j, P)], qcT[0:D, win],
                            start=False, stop=True,
                        )
                        nc.tensor.matmul(
                            sc[:, 1, :], k3[0:D, 1, ts(j, P)], qiT[0:D, win],
                            start=True, stop=True,
                        )
                    # deferred attn@v, four j-tiles behind (cross-chunk);
                    # in the last chunk's final iterations drain two per
                    # cadence so the post-exp flush shrinks to one j-tile
                    npop = 1
                    for _k in range(npop):
                        if len(pend) < 4:
                            continue
                        pem, pj, poDS = pend.pop(0)
                        for s in range(NSUB):
                            nc.tensor.matmul(
                                poDS[:, 0, s, 0:VE], pem[:, 0, ts(s, P)],
                                v_e[:, pj, :],
                                start=(pj == 0 and s == 0),
                                stop=(pj == NJT - 1),
                                skip_group_check=True,
                            )
                            nc.tensor.matmul(
                                poDS[:, 1, s, 0:VE], pem[:, 1, ts(s, P)],
                                vs_e[:, pj, :],
                                start=(pj == 0 and s == 0),
                                stop=(pj == NJT - 1),
                                skip_group_check=True,
                            )
                        if pj == NJT - 1:
                            # previous chunk complete: normalize + merge +
                            # transpose; s2/s3 deferred one iteration
                            for s in (0, 1):
                                merge_isub(ic - 1, poDS, s)
                            late_merge.append((ic - 1, poDS))
                    emes = work.tile([P, 2, IC], bf16, tag="e", bufs=6)
                    nc.scalar.activation(
                        emes[:], sc[:], Exp, bias=nbias[:, 0:1], scale=1.0,
                    )
                    pend.append((emes, j, oDS))
                    # interleaved prologue/projection work on the aux tag
                    if ic == 0:
                        if j == 0:
                            vself_quad(0)
                        elif j in (4, 8, 11):
                            vself_quad(min(j // 4, 2) + (j == 11))
                        elif j == 14:
                            qc_block(1)
                    else:
                        if j in (3, 6, 9, 12):
                            project(ic - 1, (3, 6, 9, 12).index(j))
                        elif j == 14 and ic < NICH - 1:
                            qc_block(ic + 1)
                if ic == NICH - 1:
                    # flush the two pending attn@v j-tiles (j-major: j14's
                    # groups run before exp15 lands; merges gate on the whole
                    # oDS tile anyway, so s-major interleave buys nothing)
                    for pem, pj, poDS in pend:
                        for s in range(NSUB):
                            nc.tensor.matmul(
                                poDS[:, 0, s, 0:VE], pem[:, 0, ts(s, P)],
                                v_e[:, pj, :],
                                start=False, stop=(pj == NJT - 1),
                                skip_group_check=True,
                            )
                            nc.tensor.matmul(
                                poDS[:, 1, s, 0:VE], pem[:, 1, ts(s, P)],
                                vs_e[:, pj, :],
                                start=False, stop=(pj == NJT - 1),
                                skip_group_check=True,
                            )
                    pend = []
                    # one batched 8-way reciprocal; normalize split across
                    # ACT (path-0 via scaled copy) + DVE (path-1 mult-add)
                    rc8 = work.tile([P, 2, NSUB], f32, tag="rc8", bufs=1)
                    with tc.high_priority(offset=64):
                        nc.vector.reciprocal_approx_fast(
                            out=rc8[:, :, :], in_=oDS[:, :, :, D]
                        )
                    for s in range(NSUB):
                        slot = (ic % 2) * NSUB + s
                        eng = nc.vector
                        eng.tensor_scalar(
                            mgs[slot][:, 0:D], oDS[:, 0, s, 0:D],
                            rc8[:, 0, s : s + 1], None, Alu.mult,
                        )
                        eng.scalar_tensor_tensor(
                            mgs[slot][:, 0:D], oDS[:, 1, s, 0:D],
                            rc8[:, 1, s : s + 1], mgs[slot][:, 0:D],
                            Alu.mult, Alu.add,
                        )
                    # PRIVATE psum region per s (sc ring x2, aux, freed oDS
                    # banks) so nothing serializes on buffer reuse; stage-
                    # major order: all PE transposes, all ACT treg evacs,
                    # all projects, then lo/hi fsb evacs split DVE/ACT
                    regions = []
                    for s in range(NSUB):
                        if s == 2:
                            flat = aux_tile()
                            regions.append((
                                flat[:, 0:IC], flat[:, IC : IC + C - IC],
                                flat[:, IC + 384 : IC + 448].bitcast(bf16),
                            ))
                        elif s == 3:
                            od2 = pm.tile([P, 2, NSUB, P], f32, tag="o",
                                          bufs=1)
                            regions.append((
                                od2[:, 1, :, :], od2[:, 0, 1, :],
                                od2[:, 0, 0, 0:64].bitcast(bf16),
                            ))
                        else:
                            tal = pm.tile([P, 2, IC], f32, tag="sc", bufs=2)
                            regions.append((
                                tal[:, 0, :], tal[:, 1, 0 : C - IC],
                                tal[:, 1, 384:448].bitcast(bf16),
                            ))
                    for s in range(NSUB):
                        slot = (ic % 2) * NSUB + s
                        nc.tensor.matmul(
                            regions[s][2], mgs[slot][:], ident[:],
                            start=True, stop=True, is_transpose=True,
                            skip_group_check=True,
                        )
                    for s in range(NSUB):
                        nc.scalar.copy(
                            mergedT[:, ts(NSUB * ic + s, P)], regions[s][2]
                        )
                    for s in range(NSUB):
                        nt = 4 * ic + s
                        nc.tensor.matmul(
                            regions[s][0], mergedT[:, ts(nt, P)],
                            WoT[:, 0:IC],
                            start=True, stop=True, skip_group_check=True,
                        )
                        nc.tensor.matmul(
                            regions[s][1], mergedT[:, ts(nt, P)],
                            WoT[:, IC:C],
                            start=True, stop=True, skip_group_check=True,
                        )
                    for s in range(NSUB):
                        nt = 4 * ic + s
                        lo, hi = regions[s][0], regions[s][1]
                        if s == 3:
                            # last tile: lo/hi on separate tiles + engines
                            # in parallel, two DMAs so the final (gating)
                            # transfer is the small 128-col one
                            f3lo = fout.tile([P, IC], bf16, tag="f3lo")
                            f3hi = fout.tile([P, C - IC], bf16, tag="f3hi")
                            # hi (DVE) emitted before lo (ACT): waits are
                            # quantized to engine counts at emission, so
                            # this order keeps the DVE copy off the ACT
                            # copy's completion
                            nc.vector.tensor_copy(f3hi[:], hi)
                            nc.scalar.copy(f3lo[:], lo)
                            nc.sync.dma_start(
                                out_d.ap()[ts(nt, P), IC:C], f3hi[:]
                            )
                            nc.sync.dma_start(
                                out_d.ap()[ts(nt, P), 0:IC], f3lo[:]
                            )
                            continue
                        fsb = fout.tile([P, C], bf16, tag="fsb", bufs=4)
                        # single-engine per fsb tile (cross-engine writes to
                        # one tile serialize); spread tiles DVE/ACT/Pool
                        if s % 2 == 0:
                            nc.vector.tensor_copy(fsb[:, 0:IC], lo)
                            nc.vector.tensor_copy(fsb[:, IC:C], hi)
                        else:
                            nc.scalar.copy(fsb[:, 0:IC], lo)
                            nc.scalar.copy(fsb[:, IC:C], hi)
                        nc.sync.dma_start(out_d.ap()[ts(nt, P), :], fsb[:])

    nc.compile()
    return nc


def _get_nc():
    if "nc" not in _CACHE:
        _CACHE["nc"] = _build_nc()
    return _CACHE["nc"]


def _prep_core(h, x, q_inj, k_inj, k_ref, k_refL, v_ref, Wq, Wv, Wout, bout):
    sl = slice(h * D, (h + 1) * D)

    xT = np.ascontiguousarray(x[0].T).reshape(5, P, N).transpose(1, 0, 2)

    ve = np.zeros((P, NJT, VE), BF16)
    ve[:, :, 0:D] = v_ref[h].reshape(NJT, P, D).transpose(1, 0, 2).astype(BF16)
    ve[:, :, D] = np.float32(1.0 / GAMMA)

    WoT = np.zeros((P, C), BF16)
    WoT[0:D, :] = Wout[:, sl].T.astype(BF16)
    if h == 0:
        WoT[96, :] = bout.astype(BF16)

    return {
        "xT": np.ascontiguousarray(xT).astype(BF16),
        "qiT": np.ascontiguousarray(q_inj[h].T).astype(BF16),
        "k3": np.ascontiguousarray(np.stack([
            k_refL[h].T * (GAMMA * SCALE),
            k_inj[h].T * SCALE,
            k_ref[h].T * ((1.0 - GAMMA) * SCALE),
        ], axis=1)).astype(BF16),
        "ve": ve,
        "WqT": np.ascontiguousarray(
            Wq[sl, :].T.reshape(5, P, D).transpose(1, 0, 2)).astype(BF16),
        "WvT": np.ascontiguousarray(
            Wv[sl, :].T.reshape(5, P, D).transpose(1, 0, 2)).astype(BF16),
        "WoT": WoT,
        "ident": np.eye(P, dtype=np.float32).astype(BF16),
    }


def kernel(x, q_inj, k_inj, k_ref, k_refL, v_ref, Wq, Wv, Wout, bout):
    global LAST_EXEC_NS
    f = np.float32
    args = [np.asarray(a, f) for a in
            (x, q_inj, k_inj, k_ref, k_refL, v_ref, Wq, Wv, Wout, bout)]

    nc = _get_nc()
    in_maps = [_prep_core(h, *args) for h in range(NCORES)]

    from concourse.bass_utils import run_bass_kernel_spmd

    trace = bool(os.environ.get("TRN_TRACE"))
    try:
        res = run_bass_kernel_spmd(
            nc, in_maps, core_ids=list(range(NCORES)), trace=trace
        )
    except ModuleNotFoundError:
        res = run_bass_kernel_spmd(
            nc, in_maps, core_ids=list(range(NCORES)), trace=False
        )
    LAST_EXEC_NS = res.exec_time_ns
    out = np.zeros((N, C), f)
    for r in res.results:
        out += np.asarray(r["out"], f)
    return out.reshape(1, N, C)

